# revision 10
# baseline (speedup 1.0000x reference)
"""Trainium2 Bass kernel for nn_ProbsNet.

Computation (reference):
    base = relu(BEV_p) * BEV[0]
    sig_s = sigmoid(B * (base + ST_s))                  # (4, M)
    tmp_s = einsum('im,imp->ip', sig_s, W_s).ravel()    # (84,)
    P = vmap(calc_probs)(softmax(probs_params))         # (5, 84)
    out  = mean([P[0]@tmp0, P[1]@tmp1, ..., P[4]@tmp1])

Strategy: the only heavy part is streaming the two Weight tensors
(2 x 4 x 500000 x 21 f32 = 336 MB) and reducing over m.  Shard m across
8 NeuronCores (62500 each, padded to 62592 = 128*489).  Per core, use a
dense partition-major layout: partition p owns 489 contiguous m rows for
each of the 8 (s, g) streams.  Sigmoid on the ACT engine; then 489
accumulating PE matmuls with stationary = 8 sigmoid columns [128 x 8]
and moving = W slice [128 x (8*21)], accumulating a [8 x 168]
cross-product in PSUM whose diagonal 21-blocks are the 8 per-stream
partial matvecs.  Host sums partials over cores and finishes the tiny
84-element probs math in numpy.
"""

import numpy as np

M_TOT = 500000
N_CORES = 8
M_LOC = M_TOT // N_CORES          # 62500 per core
J = 489                           # m rows per partition (padded)
M_PAD = 128 * J                   # 62592
NP = 21                           # matvec output cols per group
G = 4                             # groups
NS = 2                            # ST0/ST1 streams
C = NS * G                        # 8 combined streams
SECT = J * NP                     # 10269 floats per partition per stream
JT = 96                           # m-rows-per-partition per W supertile

TRACE = False                     # set by test harness for profiling
VERBOSE = False
LAST_RESULT = None


def _build_bass():
    import concourse.mybir as mybir
    import concourse.tile as tile
    from concourse import bacc

    nc = bacc.Bacc("TRN2", target_bir_lowering=False, debug=False)
    f32 = mybir.dt.float32
    f16 = mybir.dt.float16

    st_d = nc.dram_tensor("st", (NS, G, 128, J), f16, kind="ExternalInput")
    w_d = nc.dram_tensor("w", (NS, G, 128, SECT), f16, kind="ExternalInput")
    bias_d = nc.dram_tensor("bias", (128, 1), f32, kind="ExternalInput")
    scale_d = nc.dram_tensor("scale", (128, 1), f32, kind="ExternalInput")
    out_d = nc.dram_tensor("out", (C, C * NP), f32, kind="ExternalOutput")

    # ramp-up schedule: small first supertiles so PE starts early
    tiles = []
    jj = 0
    for jt in (32, 64):
        tiles.append((jj, jt))
        jj += jt
    P0 = jj  # sigmoid piece boundary: first piece covers the ramp tiles
    while jj < J:
        jt = min(JT, J - jj)
        tiles.append((jj, jt))
        jj += jt

    with tile.TileContext(nc) as tc:
        with (
            tc.tile_pool(name="stp", bufs=1) as stpool,
            tc.tile_pool(name="sigp", bufs=1) as sigpool,
            tc.tile_pool(name="wp", bufs=3) as wpool,
            tc.tile_pool(name="psum", bufs=1, space="PSUM") as psump,
            tc.tile_pool(name="outp", bufs=1) as outpool,
        ):
            scal = stpool.tile([128, 2], f32)
            nc.gpsimd.dma_start(out=scal[:, 0:1], in_=bias_d[:, :])
            nc.gpsimd.dma_start(out=scal[:, 1:2], in_=scale_d[:, :])

            st_a = stpool.tile([128, NS, G, P0], f16)
            st_b = stpool.tile([128, NS, G, J - P0], f16)
            sig_a = sigpool.tile([128, C, P0], f16)
            sig_b = sigpool.tile([128, C, J - P0], f16)
            for s in range(NS):
                for g in range(G):
                    nc.sync.dma_start(out=st_a[:, s, g, :], in_=st_d[s, g, :, :P0])
            for s in range(NS):
                for g in range(G):
                    nc.sync.dma_start(out=st_b[:, s, g, :], in_=st_d[s, g, :, P0:])
            nc.scalar.activation(
                sig_a[:, :, :],
                st_a[:, :, :, :],
                mybir.ActivationFunctionType.Sigmoid,
                bias=scal[:, 0:1],
                scale=scal[:, 1:2],
            )
            nc.scalar.activation(
                sig_b[:, :, :],
                st_b[:, :, :, :],
                mybir.ActivationFunctionType.Sigmoid,
                bias=scal[:, 0:1],
                scale=scal[:, 1:2],
            )

            psum_t = psump.tile([C, C * NP], f32)
            mm = 0
            for jj0, jt in tiles:
                wt = wpool.tile([128, C, JT * NP], f16)
                for s in range(NS):
                    for g in range(G):
                        nc.sync.dma_start(
                            out=wt[:, s * G + g, : jt * NP],
                            in_=w_d[s, g, :, jj0 * NP : (jj0 + jt) * NP],
                        )
                for jl in range(jt):
                    jj = jj0 + jl
                    lhsT = (
                        sig_a[:, :, jj] if jj < P0 else sig_b[:, :, jj - P0]
                    )
                    nc.tensor.matmul(
                        psum_t[:, :],
                        lhsT,
                        wt[:, :, jl * NP : (jl + 1) * NP],
                        start=(mm == 0),
                        stop=(mm == J - 1),
                    )
                    mm += 1

            out_t = outpool.tile([C, C * NP], f32)
            nc.vector.tensor_copy(out_t[:, :], psum_t[:, :])
            nc.sync.dma_start(out=out_d[:, :], in_=out_t[:, :])

    nc.compile()
    return nc


def _calc_probs_np(p):
    # p: softmaxed 4-vector -> 84-entry nested-product vector
    o2 = p[:, None] * p[None, :]
    o3 = o2[:, :, None] * p[None, None, :]
    block = np.concatenate([o2[:, :, None], o3], axis=2)          # (4,4,5)
    per_i = np.concatenate([p[:, None], block.reshape(4, 20)], axis=1)
    return per_i.reshape(-1)


def kernel(BEV, ST0, Weight0, ST1, Weight1, probs_params, BEV_p, B):
    global LAST_RESULT
    import time as _time

    _t0 = _time.time()

    def _log(msg):
        if VERBOSE:
            print(f"[kernel {_time.time() - _t0:6.1f}s] {msg}", flush=True)

    from concourse import bass_utils

    BEV = np.asarray(BEV, np.float32)
    B_f = np.float32(B)
    base = max(np.float32(BEV_p), np.float32(0.0)) * BEV[0]
    bias_v = np.full((128, 1), np.float32(B_f * base), np.float32)
    scale_v = np.full((128, 1), B_f, np.float32)

    sts = (
        np.asarray(ST0, np.float32).astype(np.float16),
        np.asarray(ST1, np.float32).astype(np.float16),
    )
    ws = (
        np.asarray(Weight0, np.float32).astype(np.float16),
        np.asarray(Weight1, np.float32).astype(np.float16),
    )

    in_maps = []
    for k in range(N_CORES):
        sl = slice(k * M_LOC, (k + 1) * M_LOC)
        st = np.zeros((NS, G, 128, J), np.float16)
        w = np.zeros((NS, G, 128, SECT), np.float16)
        for s in range(NS):
            st[s].reshape(G, M_PAD)[:, :M_LOC] = sts[s][:, sl]
            w[s].reshape(G, M_PAD, NP)[:, :M_LOC, :] = ws[s][:, sl, :]
        in_maps.append({"st": st, "w": w, "bias": bias_v, "scale": scale_v})
    _log("shards built")

    nc = _build_bass()
    _log("bass built+compiled")
    res = bass_utils.run_bass_kernel_spmd(
        nc, in_maps, core_ids=list(range(N_CORES)), trace=TRACE
    )
    _log("hw run done")
    LAST_RESULT = res

    acc = np.zeros((C, C * NP), np.float32)
    for r in res.results:
        acc += r["out"]
    tmp = np.zeros((NS, G * NP), np.float32)
    for s in range(NS):
        for g in range(G):
            c = s * G + g
            tmp[s, g * NP : (g + 1) * NP] = acc[c, c * NP : (c + 1) * NP]

    pp = np.asarray(probs_params, np.float32)
    e = np.exp(pp - pp.max(axis=1, keepdims=True))
    sm = (e / e.sum(axis=1, keepdims=True)).astype(np.float32)
    P = np.stack([_calc_probs_np(p) for p in sm]).astype(np.float32)   # (5, 84)

    outs = np.concatenate(
        [np.array([P[0] @ tmp[0]], np.float32), (P[1:] @ tmp[1]).astype(np.float32)]
    )
    return np.array(outs.mean(), dtype=np.float32)


# revision 11
# speedup vs baseline: 1.0266x; 1.0266x over previous
"""Trainium2 Bass kernel for nn_ProbsNet.

Computation (reference):
    base = relu(BEV_p) * BEV[0]
    sig_s = sigmoid(B * (base + ST_s))                  # (4, M)
    tmp_s = einsum('im,imp->ip', sig_s, W_s).ravel()    # (84,)
    P = vmap(calc_probs)(softmax(probs_params))         # (5, 84)
    out  = mean([P[0]@tmp0, P[1]@tmp1, ..., P[4]@tmp1])

Strategy: the only heavy part is streaming the two Weight tensors
(2 x 4 x 500000 x 21 f32 = 336 MB) and reducing over m.  Shard m across
8 NeuronCores (62500 each, padded to 62592 = 128*489).  Per core, use a
dense partition-major layout: partition p owns 489 contiguous m rows for
each of the 8 (s, g) streams.  Sigmoid on the ACT engine; then 489
accumulating PE matmuls with stationary = 8 sigmoid columns [128 x 8]
and moving = W slice [128 x (8*21)], accumulating a [8 x 168]
cross-product in PSUM whose diagonal 21-blocks are the 8 per-stream
partial matvecs.  Host sums partials over cores and finishes the tiny
84-element probs math in numpy.
"""

import numpy as np

M_TOT = 500000
N_CORES = 8
M_LOC = M_TOT // N_CORES          # 62500 per core
J = 489                           # m rows per partition (padded)
M_PAD = 128 * J                   # 62592
NP = 21                           # matvec output cols per group
G = 4                             # groups
NS = 2                            # ST0/ST1 streams
C = NS * G                        # 8 combined streams
SECT = J * NP                     # 10269 floats per partition per stream
JT = 96                           # m-rows-per-partition per W supertile

TRACE = False                     # set by test harness for profiling
VERBOSE = False
LAST_RESULT = None


def _build_bass():
    import concourse.mybir as mybir
    import concourse.tile as tile
    from concourse import bacc

    nc = bacc.Bacc("TRN2", target_bir_lowering=False, debug=False)
    f32 = mybir.dt.float32
    f16 = mybir.dt.float16

    st_d = nc.dram_tensor("st", (NS, G, 128, J), f16, kind="ExternalInput")
    w_d = nc.dram_tensor("w", (NS, G, 128, SECT), f16, kind="ExternalInput")
    bias_d = nc.dram_tensor("bias", (128, 1), f32, kind="ExternalInput")
    scale_d = nc.dram_tensor("scale", (128, 1), f32, kind="ExternalInput")
    out_d = nc.dram_tensor("out", (C, C * NP), f32, kind="ExternalOutput")

    # supertile schedule: small ramp-up head (PE starts early) and a
    # tapered tail (last tile's matmuls barely outlive the last DMA byte)
    sizes = [32, 64, 96, 96, 96, 89, 16]
    assert sum(sizes) == J
    tiles = []
    jj = 0
    for jt in sizes:
        tiles.append((jj, jt))
        jj += jt
    P0 = 192  # sigmoid piece boundary: first piece covers the ramp tiles

    with tile.TileContext(nc) as tc:
        with (
            tc.tile_pool(name="stp", bufs=1) as stpool,
            tc.tile_pool(name="sigp", bufs=1) as sigpool,
            tc.tile_pool(name="wp", bufs=3) as wpool,
            tc.tile_pool(name="psum", bufs=1, space="PSUM") as psump,
            tc.tile_pool(name="outp", bufs=1) as outpool,
        ):
            scal = stpool.tile([128, 2], f32)
            nc.gpsimd.dma_start(out=scal[:, 0:1], in_=bias_d[:, :])
            nc.gpsimd.dma_start(out=scal[:, 1:2], in_=scale_d[:, :])

            st_a = stpool.tile([128, NS, G, P0], f16)
            st_b = stpool.tile([128, NS, G, J - P0], f16)
            sig_a = sigpool.tile([128, C, P0], f16)
            sig_b = sigpool.tile([128, C, J - P0], f16)
            nc.scalar.dma_start(
                out=st_a[:, :, :, :],
                in_=st_d[:, :, :, :P0].rearrange("s g p j -> p s g j"),
            )
            nc.scalar.dma_start(
                out=st_b[:, :, :, :],
                in_=st_d[:, :, :, P0:].rearrange("s g p j -> p s g j"),
            )
            nc.scalar.activation(
                sig_a[:, :, :],
                st_a[:, :, :, :],
                mybir.ActivationFunctionType.Sigmoid,
                bias=scal[:, 0:1],
                scale=scal[:, 1:2],
            )
            nc.scalar.activation(
                sig_b[:, :, :],
                st_b[:, :, :, :],
                mybir.ActivationFunctionType.Sigmoid,
                bias=scal[:, 0:1],
                scale=scal[:, 1:2],
            )

            psum_t = psump.tile([C, C * NP], f32)
            mm = 0
            for jj0, jt in tiles:
                wt = wpool.tile([128, C, JT * NP], f16)
                nc.sync.dma_start(
                    out=wt[:, :, : jt * NP],
                    in_=w_d[:, :, :, jj0 * NP : (jj0 + jt) * NP].rearrange(
                        "s g p f -> p s g f"
                    ),
                )
                for jl in range(jt):
                    jj = jj0 + jl
                    lhsT = (
                        sig_a[:, :, jj] if jj < P0 else sig_b[:, :, jj - P0]
                    )
                    nc.tensor.matmul(
                        psum_t[:, :],
                        lhsT,
                        wt[:, :, jl * NP : (jl + 1) * NP],
                        start=(mm == 0),
                        stop=(mm == J - 1),
                    )
                    mm += 1

            out_t = outpool.tile([C, C * NP], f32)
            nc.vector.tensor_copy(out_t[:, :], psum_t[:, :])
            nc.sync.dma_start(out=out_d[:, :], in_=out_t[:, :])

    nc.compile()
    return nc


def _calc_probs_np(p):
    # p: softmaxed 4-vector -> 84-entry nested-product vector
    o2 = p[:, None] * p[None, :]
    o3 = o2[:, :, None] * p[None, None, :]
    block = np.concatenate([o2[:, :, None], o3], axis=2)          # (4,4,5)
    per_i = np.concatenate([p[:, None], block.reshape(4, 20)], axis=1)
    return per_i.reshape(-1)


def kernel(BEV, ST0, Weight0, ST1, Weight1, probs_params, BEV_p, B):
    global LAST_RESULT
    import time as _time

    _t0 = _time.time()

    def _log(msg):
        if VERBOSE:
            print(f"[kernel {_time.time() - _t0:6.1f}s] {msg}", flush=True)

    from concourse import bass_utils

    BEV = np.asarray(BEV, np.float32)
    B_f = np.float32(B)
    base = max(np.float32(BEV_p), np.float32(0.0)) * BEV[0]
    bias_v = np.full((128, 1), np.float32(B_f * base), np.float32)
    scale_v = np.full((128, 1), B_f, np.float32)

    sts = (
        np.asarray(ST0, np.float32).astype(np.float16),
        np.asarray(ST1, np.float32).astype(np.float16),
    )
    ws = (
        np.asarray(Weight0, np.float32).astype(np.float16),
        np.asarray(Weight1, np.float32).astype(np.float16),
    )

    in_maps = []
    for k in range(N_CORES):
        sl = slice(k * M_LOC, (k + 1) * M_LOC)
        st = np.zeros((NS, G, 128, J), np.float16)
        w = np.zeros((NS, G, 128, SECT), np.float16)
        for s in range(NS):
            st[s].reshape(G, M_PAD)[:, :M_LOC] = sts[s][:, sl]
            w[s].reshape(G, M_PAD, NP)[:, :M_LOC, :] = ws[s][:, sl, :]
        in_maps.append({"st": st, "w": w, "bias": bias_v, "scale": scale_v})
    _log("shards built")

    nc = _build_bass()
    _log("bass built+compiled")
    res = bass_utils.run_bass_kernel_spmd(
        nc, in_maps, core_ids=list(range(N_CORES)), trace=TRACE
    )
    _log("hw run done")
    LAST_RESULT = res

    acc = np.zeros((C, C * NP), np.float32)
    for r in res.results:
        acc += r["out"]
    tmp = np.zeros((NS, G * NP), np.float32)
    for s in range(NS):
        for g in range(G):
            c = s * G + g
            tmp[s, g * NP : (g + 1) * NP] = acc[c, c * NP : (c + 1) * NP]

    pp = np.asarray(probs_params, np.float32)
    e = np.exp(pp - pp.max(axis=1, keepdims=True))
    sm = (e / e.sum(axis=1, keepdims=True)).astype(np.float32)
    P = np.stack([_calc_probs_np(p) for p in sm]).astype(np.float32)   # (5, 84)

    outs = np.concatenate(
        [np.array([P[0] @ tmp[0]], np.float32), (P[1:] @ tmp[1]).astype(np.float32)]
    )
    return np.array(outs.mean(), dtype=np.float32)


# revision 12
# speedup vs baseline: 1.0750x; 1.0471x over previous
"""Trainium2 Bass kernel for nn_ProbsNet.

Computation (reference):
    base = relu(BEV_p) * BEV[0]
    sig_s = sigmoid(B * (base + ST_s))                  # (4, M)
    tmp_s = einsum('im,imp->ip', sig_s, W_s).ravel()    # (84,)
    P = vmap(calc_probs)(softmax(probs_params))         # (5, 84)
    out  = mean([P[0]@tmp0, P[1]@tmp1, ..., P[4]@tmp1])

Strategy: the heavy part is streaming the two Weight tensors
(2 x 4 x 500000 x 21 = 336 MB) and reducing over m.  Shard m across 8
NeuronCores (62500 each, padded to 62592 = 128*489) and stream in fp16
(host-cast).  Per core, dense partition-major layout: partition p owns
489 contiguous m rows for each of the 8 (s, g) streams.  The tiny
sigmoid input (1 MB/core) is precomputed on host into fp16, so the
device kernel is pure DMA + PE: 489 accumulating matmuls with
stationary = 8 sigmoid columns [128 x 8] and moving = W slice
[128 x (8*21)], accumulating a [8 x 168] cross-product in PSUM whose
diagonal 21-blocks are the 8 per-stream partial matvecs.  Host sums
partials over cores and finishes the tiny 84-element probs math.
"""

import numpy as np

M_TOT = 500000
N_CORES = 8
M_LOC = M_TOT // N_CORES          # 62500 per core
J = 489                           # m rows per partition (padded)
M_PAD = 128 * J                   # 62592
NP = 21                           # matvec output cols per group
G = 4                             # groups
NS = 2                            # ST0/ST1 streams
C = NS * G                        # 8 combined streams
SECT = J * NP                     # 10269 elems per partition per stream
JT = 96                           # max m-rows-per-partition per W supertile

TRACE = False                     # set by test harness for profiling
VERBOSE = False
LAST_RESULT = None


def _build_bass():
    import concourse.mybir as mybir
    import concourse.tile as tile
    from concourse import bacc

    nc = bacc.Bacc("TRN2", target_bir_lowering=False, debug=False)
    f32 = mybir.dt.float32
    f16 = mybir.dt.float16

    sig_d = nc.dram_tensor("sig", (128, J, C), f16, kind="ExternalInput")
    w_d = nc.dram_tensor("w", (NS, G, 128, SECT), f16, kind="ExternalInput")
    out_d = nc.dram_tensor("out", (C, C * NP), f32, kind="ExternalOutput")

    # supertile schedule: small ramp-up head (PE starts early) and a
    # tapered tail (last tile's matmuls barely outlive the last DMA byte)
    sizes = [32, 64, 96, 96, 96, 89, 16]
    assert sum(sizes) == J
    tiles = []
    jj = 0
    for jt in sizes:
        tiles.append((jj, jt))
        jj += jt
    P0 = 192  # sig piece boundary: first piece covers the ramp tiles

    with tile.TileContext(nc) as tc:
        with (
            tc.tile_pool(name="sigp", bufs=1) as sigpool,
            tc.tile_pool(name="wp", bufs=3) as wpool,
            tc.tile_pool(name="psum", bufs=1, space="PSUM") as psump,
            tc.tile_pool(name="outp", bufs=1) as outpool,
        ):
            sig_a = sigpool.tile([128, P0, C], f16)
            sig_b = sigpool.tile([128, J - P0, C], f16)
            nc.scalar.dma_start(out=sig_a[:, :, :], in_=sig_d[:, :P0, :])
            nc.scalar.dma_start(out=sig_b[:, :, :], in_=sig_d[:, P0:, :])

            psum_t = psump.tile([C, C * NP], f32)
            mm = 0
            for jj0, jt in tiles:
                wt = wpool.tile([128, C, JT * NP], f16)
                nc.sync.dma_start(
                    out=wt[:, :, : jt * NP],
                    in_=w_d[:, :, :, jj0 * NP : (jj0 + jt) * NP].rearrange(
                        "s g p f -> p s g f"
                    ),
                )
                for jl in range(jt):
                    jj = jj0 + jl
                    lhsT = (
                        sig_a[:, jj, :] if jj < P0 else sig_b[:, jj - P0, :]
                    )
                    nc.tensor.matmul(
                        psum_t[:, :],
                        lhsT,
                        wt[:, :, jl * NP : (jl + 1) * NP],
                        start=(mm == 0),
                        stop=(mm == J - 1),
                    )
                    mm += 1

            out_t = outpool.tile([C, C * NP], f32)
            nc.vector.tensor_copy(out_t[:, :], psum_t[:, :])
            nc.sync.dma_start(out=out_d[:, :], in_=out_t[:, :])

    nc.compile()
    return nc


def _calc_probs_np(p):
    # p: softmaxed 4-vector -> 84-entry nested-product vector
    o2 = p[:, None] * p[None, :]
    o3 = o2[:, :, None] * p[None, None, :]
    block = np.concatenate([o2[:, :, None], o3], axis=2)          # (4,4,5)
    per_i = np.concatenate([p[:, None], block.reshape(4, 20)], axis=1)
    return per_i.reshape(-1)


def kernel(BEV, ST0, Weight0, ST1, Weight1, probs_params, BEV_p, B):
    global LAST_RESULT
    import time as _time

    _t0 = _time.time()

    def _log(msg):
        if VERBOSE:
            print(f"[kernel {_time.time() - _t0:6.1f}s] {msg}", flush=True)

    from concourse import bass_utils

    BEV = np.asarray(BEV, np.float32)
    B_f = np.float32(B)
    base = max(np.float32(BEV_p), np.float32(0.0)) * BEV[0]

    # host-side sigmoid (1.2% of the data volume; keeps the device kernel
    # a pure DMA+matmul stream) computed in f32, stored fp16
    sigs = []
    for STs in (ST0, ST1):
        x = B_f * (base + np.asarray(STs, np.float32))
        sigs.append((1.0 / (1.0 + np.exp(-x))).astype(np.float16))

    ws = (
        np.asarray(Weight0, np.float32).astype(np.float16),
        np.asarray(Weight1, np.float32).astype(np.float16),
    )

    in_maps = []
    for k in range(N_CORES):
        sl = slice(k * M_LOC, (k + 1) * M_LOC)
        sig = np.zeros((NS, G, 128, J), np.float16)
        w = np.zeros((NS, G, 128, SECT), np.float16)
        for s in range(NS):
            sig[s].reshape(G, M_PAD)[:, :M_LOC] = sigs[s][:, sl]
            w[s].reshape(G, M_PAD, NP)[:, :M_LOC, :] = ws[s][:, sl, :]
        # device wants sig as [p, j, c] with c = s*4+g
        sig_pjc = np.ascontiguousarray(sig.reshape(C, 128, J).transpose(1, 2, 0))
        in_maps.append({"sig": sig_pjc, "w": w})
    _log("shards built")

    nc = _build_bass()
    _log("bass built+compiled")
    res = bass_utils.run_bass_kernel_spmd(
        nc, in_maps, core_ids=list(range(N_CORES)), trace=TRACE
    )
    _log("hw run done")
    LAST_RESULT = res

    acc = np.zeros((C, C * NP), np.float32)
    for r in res.results:
        acc += r["out"]
    tmp = np.zeros((NS, G * NP), np.float32)
    for s in range(NS):
        for g in range(G):
            c = s * G + g
            tmp[s, g * NP : (g + 1) * NP] = acc[c, c * NP : (c + 1) * NP]

    pp = np.asarray(probs_params, np.float32)
    e = np.exp(pp - pp.max(axis=1, keepdims=True))
    sm = (e / e.sum(axis=1, keepdims=True)).astype(np.float32)
    P = np.stack([_calc_probs_np(p) for p in sm]).astype(np.float32)   # (5, 84)

    outs = np.concatenate(
        [np.array([P[0] @ tmp[0]], np.float32), (P[1:] @ tmp[1]).astype(np.float32)]
    )
    return np.array(outs.mean(), dtype=np.float32)


# revision 13
# speedup vs baseline: 1.0773x; 1.0022x over previous
"""Trainium2 Bass kernel for nn_ProbsNet.

Computation (reference):
    base = relu(BEV_p) * BEV[0]
    sig_s = sigmoid(B * (base + ST_s))                  # (4, M)
    tmp_s = einsum('im,imp->ip', sig_s, W_s).ravel()    # (84,)
    P = vmap(calc_probs)(softmax(probs_params))         # (5, 84)
    out  = mean([P[0]@tmp0, P[1]@tmp1, ..., P[4]@tmp1])

Strategy: the heavy part is streaming the two Weight tensors
(2 x 4 x 500000 x 21 = 336 MB) and reducing over m.  Shard m across 8
NeuronCores (62500 each, padded to 62592 = 128*489) and stream in fp16
(host-cast).  Per core, dense partition-major layout: partition p owns
489 contiguous m rows for each of the 8 (s, g) streams.  The tiny
sigmoid input (1 MB/core) is precomputed on host into fp16, so the
device kernel is pure DMA + PE: 489 accumulating matmuls with
stationary = 8 sigmoid columns [128 x 8] and moving = W slice
[128 x (8*21)], accumulating a [8 x 168] cross-product in PSUM whose
diagonal 21-blocks are the 8 per-stream partial matvecs.  Host sums
partials over cores and finishes the tiny 84-element probs math.
"""

import numpy as np

M_TOT = 500000
N_CORES = 8
M_LOC = M_TOT // N_CORES          # 62500 per core
J = 489                           # m rows per partition (padded)
M_PAD = 128 * J                   # 62592
NP = 21                           # matvec output cols per group
G = 4                             # groups
NS = 2                            # ST0/ST1 streams
C = NS * G                        # 8 combined streams
SECT = J * NP                     # 10269 elems per partition per stream
JT = 96                           # max m-rows-per-partition per W supertile

TRACE = False                     # set by test harness for profiling
VERBOSE = False
LAST_RESULT = None


def _build_bass():
    import concourse.mybir as mybir
    import concourse.tile as tile
    from concourse import bacc

    nc = bacc.Bacc("TRN2", target_bir_lowering=False, debug=False)
    f32 = mybir.dt.float32
    f16 = mybir.dt.float16

    sig_d = nc.dram_tensor("sig", (128, J, C), f16, kind="ExternalInput")
    w_d = nc.dram_tensor("w", (NS, G, 128, SECT), f16, kind="ExternalInput")
    out_d = nc.dram_tensor("out", (C, C * NP), f32, kind="ExternalOutput")

    # supertile schedule: small ramp-up head (PE starts early) and a
    # tapered tail (last tile's matmuls barely outlive the last DMA byte)
    sizes = [32, 64, 96, 96, 96, 81, 16, 8]
    assert sum(sizes) == J
    tiles = []
    jj = 0
    for jt in sizes:
        tiles.append((jj, jt))
        jj += jt
    P0 = 192  # sig piece boundary: first piece covers the ramp tiles

    with tile.TileContext(nc) as tc:
        with (
            tc.tile_pool(name="sigp", bufs=1) as sigpool,
            tc.tile_pool(name="wp", bufs=3) as wpool,
            tc.tile_pool(name="psum", bufs=1, space="PSUM") as psump,
            tc.tile_pool(name="outp", bufs=1) as outpool,
        ):
            sig_a = sigpool.tile([128, P0, C], f16)
            sig_b = sigpool.tile([128, J - P0, C], f16)
            nc.scalar.dma_start(out=sig_a[:, :, :], in_=sig_d[:, :P0, :])
            nc.scalar.dma_start(out=sig_b[:, :, :], in_=sig_d[:, P0:, :])

            psum_t = psump.tile([C, C * NP], f32)
            mm = 0
            for jj0, jt in tiles:
                wt = wpool.tile([128, C, JT * NP], f16)
                nc.sync.dma_start(
                    out=wt[:, :, : jt * NP],
                    in_=w_d[:, :, :, jj0 * NP : (jj0 + jt) * NP].rearrange(
                        "s g p f -> p s g f"
                    ),
                )
                for jl in range(jt):
                    jj = jj0 + jl
                    lhsT = (
                        sig_a[:, jj, :] if jj < P0 else sig_b[:, jj - P0, :]
                    )
                    nc.tensor.matmul(
                        psum_t[:, :],
                        lhsT,
                        wt[:, :, jl * NP : (jl + 1) * NP],
                        start=(mm == 0),
                        stop=(mm == J - 1),
                    )
                    mm += 1

            out_t = outpool.tile([C, C * NP], f32)
            nc.vector.tensor_copy(out_t[:, :], psum_t[:, :])
            nc.sync.dma_start(out=out_d[:, :], in_=out_t[:, :])

    nc.compile()
    return nc


def _calc_probs_np(p):
    # p: softmaxed 4-vector -> 84-entry nested-product vector
    o2 = p[:, None] * p[None, :]
    o3 = o2[:, :, None] * p[None, None, :]
    block = np.concatenate([o2[:, :, None], o3], axis=2)          # (4,4,5)
    per_i = np.concatenate([p[:, None], block.reshape(4, 20)], axis=1)
    return per_i.reshape(-1)


def kernel(BEV, ST0, Weight0, ST1, Weight1, probs_params, BEV_p, B):
    global LAST_RESULT
    import time as _time

    _t0 = _time.time()

    def _log(msg):
        if VERBOSE:
            print(f"[kernel {_time.time() - _t0:6.1f}s] {msg}", flush=True)

    from concourse import bass_utils

    BEV = np.asarray(BEV, np.float32)
    B_f = np.float32(B)
    base = max(np.float32(BEV_p), np.float32(0.0)) * BEV[0]

    # host-side sigmoid (1.2% of the data volume; keeps the device kernel
    # a pure DMA+matmul stream) computed in f32, stored fp16
    sigs = []
    for STs in (ST0, ST1):
        x = B_f * (base + np.asarray(STs, np.float32))
        sigs.append((1.0 / (1.0 + np.exp(-x))).astype(np.float16))

    ws = (
        np.asarray(Weight0, np.float32).astype(np.float16),
        np.asarray(Weight1, np.float32).astype(np.float16),
    )

    in_maps = []
    for k in range(N_CORES):
        sl = slice(k * M_LOC, (k + 1) * M_LOC)
        sig = np.zeros((NS, G, 128, J), np.float16)
        w = np.zeros((NS, G, 128, SECT), np.float16)
        for s in range(NS):
            sig[s].reshape(G, M_PAD)[:, :M_LOC] = sigs[s][:, sl]
            w[s].reshape(G, M_PAD, NP)[:, :M_LOC, :] = ws[s][:, sl, :]
        # device wants sig as [p, j, c] with c = s*4+g
        sig_pjc = np.ascontiguousarray(sig.reshape(C, 128, J).transpose(1, 2, 0))
        in_maps.append({"sig": sig_pjc, "w": w})
    _log("shards built")

    nc = _build_bass()
    _log("bass built+compiled")
    res = bass_utils.run_bass_kernel_spmd(
        nc, in_maps, core_ids=list(range(N_CORES)), trace=TRACE
    )
    _log("hw run done")
    LAST_RESULT = res

    acc = np.zeros((C, C * NP), np.float32)
    for r in res.results:
        acc += r["out"]
    tmp = np.zeros((NS, G * NP), np.float32)
    for s in range(NS):
        for g in range(G):
            c = s * G + g
            tmp[s, g * NP : (g + 1) * NP] = acc[c, c * NP : (c + 1) * NP]

    pp = np.asarray(probs_params, np.float32)
    e = np.exp(pp - pp.max(axis=1, keepdims=True))
    sm = (e / e.sum(axis=1, keepdims=True)).astype(np.float32)
    P = np.stack([_calc_probs_np(p) for p in sm]).astype(np.float32)   # (5, 84)

    outs = np.concatenate(
        [np.array([P[0] @ tmp[0]], np.float32), (P[1:] @ tmp[1]).astype(np.float32)]
    )
    return np.array(outs.mean(), dtype=np.float32)
